# revision 50
# baseline (speedup 1.0000x reference)
"""KAN EncoderNetwork kernel for 8 Trainium2 NeuronCores.

Strategy (data-parallel, batch sharded 8 ways, weights replicated):

Each KAN layer  out = silu(x) @ sb + einsum('big,iog->bo', B(x), coef*ss)
is reformulated as matmuls over an expanded feature matrix: for every
128-wide input chunk the feature rows are spline basis blocks + 1 silu
block.  The uniform-grid cubic B-spline basis has the closed form
(cardinal spline, t = 2.5x + 5.5):

  6*B_g(x) = relu(2-w)^3 - 4*relu(1-w)^3,   w = |2.5x + 3.5 - g|

computed on ScalarE (Abs/Relu) + custom VectorE ops, balanced across the
two engines.  Everything stays feature-major ([feat, batch]) so layer
outputs in PSUM feed the next layer's basis computation directly; only
the final layer is transposed back (TensorE) for the [batch, out] output.

Precision/throughput structure (validated vs the fp32 reference on the
host, total rel err ~1e-2 vs the 2e-2 budget):
 * activations shrink layer by layer (x.std 0.5 -> 0.1), so deep-layer
   basis energy concentrates in the center blocks g=3,4: edge blocks are
   numerically dead and are DROPPED (L1: g 0,7; L2/L3: g 0,1,6,7).
 * center blocks g=3,4 stay bf16 on deep layers (they carry most of the
   spline signal); remaining blocks run as fp8e4m3 DoubleRow matmuls
   (2 K-tiles per instruction, ~2x PE throughput).  L0's activations are
   wide, so all 8 of its blocks tolerate fp8.
 * the silu/base block stays bf16 everywhere.
All weights are pre-scaled by ALPHA=2^15 so the tiny spline weights sit
in fp8's normal range; consumers of a layer's PSUM fold the 1/ALPHA
descale into their activation scale/s0 constants (no extra ops).
"""

import sys

sys.path.insert(0, "/opt/trn_rl_repo")

import numpy as np
import ml_dtypes

import concourse.bacc as bacc
import concourse.mybir as mybir
import concourse.tile as tile
from concourse.bass_utils import run_bass_kernel_spmd
from concourse.dve_spec import Spec, Src0, Src1, C0, C1, C2, Zero, relu, sq, maxx, lower, _has_src1
from concourse.dve_uop import DveOpSpec
from concourse.dve_ops import (
    DveOp,
    OPS,
    _SUB_OPCODE_FOR_NAME,
    CUSTOM_DVE_SPECS,
    _CUSTOM_DVE_ROW_BASE,
)

F32 = mybir.dt.float32
BF16 = mybir.dt.bfloat16
FP8 = mybir.dt.float8e4
AF = mybir.ActivationFunctionType
DR = mybir.MatmulPerfMode.DoubleRow

WIDTH = [512, 1024, 1024, 1024, 256]
NCORES = 8
BATCH = 4096
BPC = BATCH // NCORES  # 512 batch rows per core

ALPHA = float(2 ** 15)  # weight pre-scale so fp8 spline weights are normal
INV_ALPHA = 1.0 / ALPHA

# per-layer block config: fp8 DoubleRow pairs, bf16 spline blocks, and
# which blocks run on the ScalarE Abs/Relu pipeline (vb) vs all-DVE
LCFG = [
    dict(pairs=[(0, 1), (2, 3), (4, 5), (6, 7)], bf=[], vb={5, 6, 7}),
    dict(pairs=[(1, 6), (2, 5), (3, 4)], bf=[], vb={2, 3, 4}),
    dict(pairs=[(2, 5)], bf=[3, 4], vb={3, 4, 5}),
    dict(pairs=[(2, 5)], bf=[3, 4], vb={4, 5}),
]
for _c in LCFG:
    _c["np"] = len(_c["pairs"])          # DoubleRow matmuls per chunk
    _c["ns"] = len(_c["bf"]) + 1         # bf16 matmuls per chunk (+silu)
    _c["slots"] = _c["np"] + _c["ns"]    # total matmul slots per chunk


def _register_op(name, spec):
    if name in _SUB_OPCODE_FOR_NAME:
        for op in OPS:
            if op.name == name:
                return op
        raise RuntimeError(f"opcode row taken but op {name} missing")
    row = _CUSTOM_DVE_ROW_BASE + len(OPS)
    _SUB_OPCODE_FOR_NAME[name] = row
    shas = {}
    for ver in ("v3", "v4"):
        uops = lower(spec, ver=ver)
        shas[ver] = DveOpSpec(
            name=name, opcode=row, uops=uops, rd1_en=_has_src1(spec)
        ).sha(ver)
    op = DveOp(name, spec, subdim=False, uops_sha=shas)
    OPS.append(op)
    CUSTOM_DVE_SPECS[name] = spec
    return op


# out = a^3 + s1 * relu(a - s0)^3   (in0 = a2 = relu(2-w); 1 stream)
_rb = relu(Src0 - C0)
KAN_TENT_POLY = _register_op(
    "KAN_TENT_POLY",
    Spec(
        body=sq(Src0) * Src0 + sq(_rb) * _rb * C1,
        reference=lambda in0, in1, s0, s1, imm2: in0**3
        + s1 * np.maximum(in0 - s0, 0.0) ** 3,
    ),
)

# a2 = relu(imm2 - |x*s0 + s1|)    (variant E pass 1; 1 stream, from x)
_u = Src0 * C0 + C1
_wabs = maxx(_u, Zero - _u)
KAN_A2_ABS = _register_op(
    "KAN_A2_ABS",
    Spec(
        body=relu(C2 - _wabs),
        reference=lambda in0, in1, s0, s1, imm2: np.maximum(
            imm2 - np.abs(in0 * s0 + s1), 0.0
        ),
    ),
)


def _build_nc():
    nc = bacc.Bacc(trn_type="TRN2")
    xT_dr = nc.dram_tensor("xT", [WIDTH[0], BPC], F32, kind="ExternalInput")
    w8_dr = [
        nc.dram_tensor(
            f"w8_{l}",
            [(WIDTH[l] // 128) * LCFG[l]["np"] * 128, 2 * WIDTH[l + 1]],
            FP8, kind="ExternalInput",
        )
        for l in range(4)
    ]
    wb_dr = [
        nc.dram_tensor(
            f"wb_{l}",
            [(WIDTH[l] // 128) * LCFG[l]["ns"] * 128, WIDTH[l + 1]],
            BF16, kind="ExternalInput",
        )
        for l in range(4)
    ]
    bias_dr = nc.dram_tensor("bias_c", [128, 9], F32, kind="ExternalInput")
    # feature-major ALPHA-scaled bf16 output; host transposes + descales
    out_dr = nc.dram_tensor("out", [WIDTH[4], BPC], BF16,
                            kind="ExternalOutput")

    # [p, c, j, w] where w = (oc, i, o128) flat: all of a chunk's
    # DoubleRow pairs land in ONE DMA, and any oc-aligned column phase is
    # a contiguous w-slice (keeps the DMA AP rank at 3)
    w8_r = [
        w8_dr[l].rearrange("(c j p) w -> p c j w", p=128, j=LCFG[l]["np"])
        for l in range(4)
    ]
    wb_r = [wb_dr[l].rearrange("(k p) o -> p k o", p=128) for l in range(4)]

    with tile.TileContext(nc) as tc:
        with (
            tc.tile_pool(name="const", bufs=1) as const_pool,
            tc.tile_pool(name="xt", bufs=2) as xt_pool,
            tc.tile_pool(name="ft", bufs=11) as ft_pool,
            tc.tile_pool(name="wt", bufs=10) as wt_pool,
            tc.tile_pool(name="tmp", bufs=3) as tmp_pool,
            tc.tile_pool(name="outp", bufs=1) as out_pool,
            tc.tile_pool(name="psum", bufs=8, space="PSUM") as psum_pool,
        ):
            nic0 = WIDTH[0] // 128
            xT_r = xT_dr.rearrange("(c p) b -> p c b", p=128)
            # DMA order = first-use order: the sync queue drains serially
            # and the FIRST dma only lands ~10us in (queue prologue + DGE
            # pipe), so everything the startup chain needs goes first.
            xt = xt_pool.tile([128, nic0, BPC], F32, tag="xt")
            nc.sync.dma_start(xt[:, 0:1, :], xT_r[:, 0:1, :])
            # col g in 0..7: Abs bias 3.5-g ; col 8: +2.0 (variant-B Relu
            # bias).  DMA'd, not memset: gpsimd's ~5us prologue would gate
            # the first ScalarE basis ops otherwise.
            bias = const_pool.tile([128, 9], F32, tag="bias")
            nc.sync.dma_start(bias, bias_dr.ap())
            pre_wt = wt_pool.tile([128, LCFG[0]["np"], 4, 2, 128], FP8,
                                  tag="wt8pre", bufs=1, name="wt_pre")
            nc.sync.dma_start(pre_wt, w8_r[0][:, 0, :, 0:1024])
            # warm-up operand, filled by DMA (content irrelevant)
            junk = const_pool.tile([128, BPC], F32, tag="junk")
            nc.sync.dma_start(junk, xT_r[:, 0, :])
            pre_wb = wt_pool.tile([128, 1, WIDTH[1]], BF16, tag="wtb",
                                  name="wb_pre")
            nc.sync.dma_start(pre_wb, wb_r[0][:, 0:1, :])
            nc.sync.dma_start(xt[:, 1:nic0, :], xT_r[:, 1:nic0, :])

            def new_ft(l, c):
                cfg = LCFG[l]
                ft = ft_pool.tile([128, 2 * cfg["np"], BPC], FP8, tag="ft8",
                                  name=f"ft_{l}_{c}")
                fb = ft_pool.tile([128, cfg["ns"], BPC], BF16, tag="ftb",
                                  name=f"fb_{l}_{c}")
                return ft, fb

            def emit_fast_restart(l, src_psum):
                """First fp8 block of chunk 0 computed straight from the
                previous layer's PSUM (holds ALPHA*x) so the PE restarts
                quickly."""
                g0 = LCFG[l]["pairs"][0][0]
                a2 = tmp_pool.tile([128, BPC], F32, tag="wv",
                                   name=f"a2fr_{l}")
                nc.vector._custom_dve(KAN_A2_ABS, out=a2, in0=src_psum,
                                      s0=2.5 * INV_ALPHA, s1=3.5 - g0,
                                      imm2=2.0)
                ft, fb = new_ft(l, 0)
                nc.vector._custom_dve(KAN_TENT_POLY, out=ft[:, 0, :],
                                      in0=a2, s0=1.0, s1=-4.0)
                return ft, fb

            def emit_copies(xt, src_psums, chunks):
                for i, c in enumerate(chunks):
                    if i % 2 == 0:
                        nc.scalar.copy(xt[:, c, :], src_psums[c])
                    else:
                        nc.vector.tensor_copy(xt[:, c, :], src_psums[c])

            def emit_basis(l, xt, c, ft, fb, skip_first=False):
                """ft: fp8 spline pairs (slot 2j+i = block pairs[j][i]);
                fb: bf16 blocks (cfg['bf'] order, then silu).
                fp8 pairs stage their a2 in bf16 and share ONE merged
                KAN_TENT_POLY (the DR matmul consumes both slots at once,
                so merging costs no pipeline granularity); bf16 center
                blocks keep f32 a2 for precision.
                For l>0 the input xt holds ALPHA*x."""
                cfg = LCFG[l]
                ds = INV_ALPHA if l > 0 else 1.0
                xa = xt[:, c, :]

                def a2_for(g, dst):
                    if g in cfg["vb"]:
                        wv = tmp_pool.tile([128, BPC], F32, tag="wv",
                                           name=f"wv_{l}_{c}_{g}")
                        nc.scalar.activation(wv, xa, AF.Abs,
                                             bias=bias[:, g : g + 1],
                                             scale=2.5 * ds)
                        nc.scalar.activation(dst, wv, AF.Relu,
                                             bias=bias[:, 8:9], scale=-1.0)
                    else:
                        nc.vector._custom_dve(KAN_A2_ABS, out=dst, in0=xa,
                                              s0=2.5 * ds, s1=3.5 - g,
                                              imm2=2.0)

                for j, pr in enumerate(cfg["pairs"]):
                    a2p = tmp_pool.tile([128, 2, BPC], BF16, tag="a2p",
                                        name=f"a2p_{l}_{c}_{j}")
                    lo = 1 if (skip_first and j == 0) else 0
                    for i in range(lo, 2):
                        a2_for(pr[i], a2p[:, i, :])
                    nc.vector._custom_dve(
                        KAN_TENT_POLY, out=ft[:, 2 * j + lo : 2 * j + 2, :],
                        in0=a2p[:, lo:2, :], s0=1.0, s1=-4.0,
                    )
                for s, g in enumerate(cfg["bf"]):
                    a2 = tmp_pool.tile([128, BPC], F32, tag="qv",
                                       name=f"a2_{l}_{c}_{g}")
                    a2_for(g, a2)
                    nc.vector._custom_dve(KAN_TENT_POLY, out=fb[:, s, :],
                                          in0=a2, s0=1.0, s1=-4.0)
                nc.scalar.activation(fb[:, cfg["ns"] - 1, :], xa, AF.Silu,
                                     scale=ds)

            def dma_weights(l, c, ocs, col0):
                """One DMA per weight class per chunk, covering all of
                `ocs` columns (sub-phases slice the same tile)."""
                cfg = LCFG[l]
                ncol = len(ocs) * 128
                if l == 0 and c == 0 and col0 == 0:
                    return pre_wt, pre_wb
                wt = wt_pool.tile([128, cfg["np"], len(ocs), 2, 128], FP8,
                                  tag="wt8", name=f"wt_{l}_{c}_{col0}")
                nc.sync.dma_start(
                    wt, w8_r[l][:, c, :, 2 * col0 : 2 * (col0 + ncol)]
                )
                wb = wt_pool.tile([128, cfg["ns"], ncol], BF16, tag="wtb",
                                  name=f"wb_{l}_{c}_{col0}")
                nc.sync.dma_start(
                    wb, wb_r[l][:, c * cfg["ns"] : (c + 1) * cfg["ns"],
                                col0 : col0 + ncol]
                )
                return wt, wb

            def emit_mms(l, c, wt, wb, ft, fb, psums, ocs, col0, KB):
                """col0 = base column of the weight tiles."""
                cfg = LCFG[l]
                ob = col0 // 128
                for j in range(cfg["np"]):
                    kb = c * cfg["slots"] + j
                    for oc in ocs:
                        nc.tensor.matmul(
                            psums[oc], wt[:, j, oc - ob, :, :],
                            ft[:, 2 * j : 2 * j + 2, :],
                            start=(kb == 0), stop=False, perf_mode=DR,
                        )
                for s in range(cfg["ns"]):
                    kb = c * cfg["slots"] + cfg["np"] + s
                    for oc in ocs:
                        nc.tensor.matmul(
                            psums[oc],
                            wb[:, s, (oc - ob) * 128 : (oc - ob + 1) * 128],
                            fb[:, s, :],
                            start=False, stop=(kb == KB - 1),
                        )

            # ---- layer 0: out-chunk phase split (4 + 4 banks) so layer-1
            # basis production fully overlaps phase B matmuls ----
            nicl0, nocl0 = WIDTH[0] // 128, WIDTH[1] // 128
            KB0 = LCFG[0]["slots"] * nicl0
            psums0 = [
                psum_pool.tile([128, BPC], F32, tag="psum", name=f"ps_0_{i}")
                for i in range(4)
            ]

            # HAM warm-up: slow fp32 matmuls at scheduler priority 0 so
            # they sit at the head of the PE queue and keep the array busy
            # through the ~3.4us HAM window while the first basis chain
            # ramps; the real kb==0 start=True matmul clears the bank.
            with tc.high_priority():
                for wi in range(2):
                    nc.tensor.matmul(
                        psums0[3], junk[:, 0:128], junk,
                        start=True, stop=True, skip_group_check=True,
                    )
            l0_fts = []
            w0A = []
            for c in range(nicl0):
                ft, fb = new_ft(0, c)
                emit_basis(0, xt, c, ft, fb)
                l0_fts.append((ft, fb))
                w0A.append(dma_weights(0, c, [0, 1, 2, 3], 0))
                emit_mms(0, c, *w0A[c], ft, fb, psums0, [0, 1], 0, KB0)

            # banks 0,1 done: layer-1 chunks 0,1 start while L0-A finishes
            nic1, noc1 = WIDTH[1] // 128, WIDTH[2] // 128
            KB1 = LCFG[1]["slots"] * nic1
            xt1 = xt_pool.tile([128, nic1, BPC], F32, tag="xt", name="xt_1")
            ft1_0, fb1_0 = emit_fast_restart(1, psums0[0])
            emit_copies(xt1, psums0, [0, 1])
            l1_fts = [(ft1_0, fb1_0)]
            emit_basis(1, xt1, 0, ft1_0, fb1_0, skip_first=True)
            ft, fb = new_ft(1, 1)
            emit_basis(1, xt1, 1, ft, fb)
            l1_fts.append((ft, fb))

            for c in range(nicl0):
                emit_mms(0, c, *w0A[c], *l0_fts[c], psums0, [2, 3], 0, KB0)
            emit_copies(xt1, psums0, [2, 3])
            psums1 = [
                psum_pool.tile([128, BPC], F32, tag="psum", name=f"ps_1_{i}")
                for i in range(4)
            ]
            for c in (2, 3):
                ft, fb = new_ft(1, c)
                emit_basis(1, xt1, c, ft, fb)
                l1_fts.append((ft, fb))

            # layer-0 phase B: weights for banks 4-7 DMA'd once per chunk;
            # matmuls run in oc-pair sub-phases so layer-1's late input
            # chunks materialize (and their basis runs) before phase B ends
            psums0 += [
                psum_pool.tile([128, BPC], F32, tag="psum", name=f"ps_0_{i}")
                for i in range(4, nocl0)
            ]
            w0B = [dma_weights(0, c, [4, 5, 6, 7], 512)
                   for c in range(nicl0)]
            for c in range(nicl0):
                emit_mms(0, c, *w0B[c], *l0_fts[c], psums0, [4, 5], 512, KB0)
            emit_copies(xt1, psums0, [4, 5])
            for c in (4, 5):
                ft, fb = new_ft(1, c)
                emit_basis(1, xt1, c, ft, fb)
                l1_fts.append((ft, fb))
            for c in range(nicl0):
                emit_mms(0, c, *w0B[c], *l0_fts[c], psums0, [6, 7], 512, KB0)
            emit_copies(xt1, psums0, [6, 7])
            psums1 += [
                psum_pool.tile([128, BPC], F32, tag="psum", name=f"ps_1_{i}")
                for i in range(4, noc1)
            ]
            for c in (6, 7):
                ft, fb = new_ft(1, c)
                emit_basis(1, xt1, c, ft, fb)
                l1_fts.append((ft, fb))

            # keep-warm filler: bridges the feature-starved boundary so
            # HAM does not re-throttle the PE
            nc.tensor.matmul(psums1[3], junk[:, 0:128], junk,
                             start=True, stop=True, skip_group_check=True)
            # layer-1 phase A (production already done above), sub-phased
            nic2, noc2 = WIDTH[2] // 128, WIDTH[3] // 128
            KB2 = LCFG[2]["slots"] * nic2
            w1A = [dma_weights(1, c, [0, 1, 2, 3], 0) for c in range(nic1)]
            for c in range(nic1):
                emit_mms(1, c, *w1A[c], *l1_fts[c], psums1, [0, 1], 0, KB1)

            # banks 0,1 done: layer-2 chunks 0,1 start while L1-A finishes
            xt2 = xt_pool.tile([128, nic2, BPC], F32, tag="xt", name="xt_2")
            ft2_0, fb2_0 = emit_fast_restart(2, psums1[0])
            emit_copies(xt2, psums1, [0, 1])
            l2_fts = [(ft2_0, fb2_0)]
            emit_basis(2, xt2, 0, ft2_0, fb2_0, skip_first=True)
            ft, fb = new_ft(2, 1)
            emit_basis(2, xt2, 1, ft, fb)
            l2_fts.append((ft, fb))

            for c in range(nic1):
                emit_mms(1, c, *w1A[c], *l1_fts[c], psums1, [2, 3], 0, KB1)
            emit_copies(xt2, psums1, [2, 3])
            psums2 = [
                psum_pool.tile([128, BPC], F32, tag="psum", name=f"ps_2_{i}")
                for i in range(4)
            ]
            for c in (2, 3):
                ft, fb = new_ft(2, c)
                emit_basis(2, xt2, c, ft, fb)
                l2_fts.append((ft, fb))

            # layer-1 phase B (oc-pair sub-phases, one weight DMA per chunk)
            w1B = [dma_weights(1, c, [4, 5, 6, 7], 512) for c in range(nic1)]
            for c in range(nic1):
                emit_mms(1, c, *w1B[c], *l1_fts[c], psums1, [4, 5], 512, KB1)
            emit_copies(xt2, psums1, [4, 5])
            for c in (4, 5):
                ft, fb = new_ft(2, c)
                emit_basis(2, xt2, c, ft, fb)
                l2_fts.append((ft, fb))
            for c in range(nic1):
                emit_mms(1, c, *w1B[c], *l1_fts[c], psums1, [6, 7], 512, KB1)
            emit_copies(xt2, psums1, [6, 7])
            psums2 += [
                psum_pool.tile([128, BPC], F32, tag="psum", name=f"ps_2_{i}")
                for i in range(4, noc2)
            ]
            for c in (6, 7):
                ft, fb = new_ft(2, c)
                emit_basis(2, xt2, c, ft, fb)
                l2_fts.append((ft, fb))

            nc.tensor.matmul(psums2[3], junk[:, 0:128], junk,
                             start=True, stop=True, skip_group_check=True)
            # layer-2 phase A (production already done above)
            for c in range(nic2):
                wt, wb = dma_weights(2, c, [0, 1, 2, 3], 0)
                emit_mms(2, c, wt, wb, *l2_fts[c], psums2,
                         [0, 1, 2, 3], 0, KB2)

            # between phases: layer-3 input chunks 0..3 + their basis
            nic3, noc3 = WIDTH[3] // 128, WIDTH[4] // 128
            KB3 = LCFG[3]["slots"] * nic3
            xt3 = xt_pool.tile([128, nic3, BPC], F32, tag="xt", name="xt_3")
            ft3_0, fb3_0 = emit_fast_restart(3, psums2[0])
            emit_copies(xt3, psums2, range(4))
            psums3 = [
                psum_pool.tile([128, BPC], F32, tag="psum", name=f"ps_3_{i}")
                for i in range(noc3)
            ]
            l3_fts = [(ft3_0, fb3_0)]
            emit_basis(3, xt3, 0, ft3_0, fb3_0, skip_first=True)
            for c in range(1, 4):
                ft, fb = new_ft(3, c)
                emit_basis(3, xt3, c, ft, fb)
                l3_fts.append((ft, fb))

            # layer-2 phase B (oc-pair sub-phases, one weight DMA per chunk)
            w2B = [dma_weights(2, c, [4, 5, 6, 7], 512) for c in range(nic2)]
            for c in range(nic2):
                emit_mms(2, c, *w2B[c], *l2_fts[c], psums2, [4, 5], 512, KB2)
            emit_copies(xt3, psums2, [4, 5])
            for c in (4, 5):
                ft, fb = new_ft(3, c)
                emit_basis(3, xt3, c, ft, fb)
                l3_fts.append((ft, fb))
            for c in range(nic2):
                emit_mms(2, c, *w2B[c], *l2_fts[c], psums2, [6, 7], 512, KB2)
            emit_copies(xt3, psums2, [6, 7])
            for c in (6, 7):
                ft, fb = new_ft(3, c)
                emit_basis(3, xt3, c, ft, fb)
                l3_fts.append((ft, fb))

            # layer-3 matmuls, chunk-major (late chunks last, so they hit
            # the PE right after their basis lands); output stays
            # feature-major ALPHA-scaled, host transposes + descales.
            s3 = out_pool.tile([128, noc3, BPC], BF16, tag="s3")
            out_r = out_dr.rearrange("(oc p) b -> p oc b", p=128)
            for c in range(nic3):
                wt, wb = dma_weights(3, c, [0, 1], 0)
                emit_mms(3, c, wt, wb, *l3_fts[c], psums3, [0, 1], 0, KB3)
            nc.scalar.copy(s3[:, 0, :], psums3[0])
            nc.vector.tensor_copy(s3[:, 1, :], psums3[1])
            for oc in range(noc3):
                nc.sync.dma_start(out_r[:, oc : oc + 1, :],
                                  s3[:, oc : oc + 1, :])
    nc.finalize()
    return nc


_NC_CACHE = []


def _get_nc():
    if not _NC_CACHE:
        _NC_CACHE.append(_build_nc())
    return _NC_CACHE[0]


def _build_weights(inp):
    """Pre-scale by ALPHA and pack per LCFG: fp8 DoubleRow pairs as
    [(c j p), (i o)] (i = which of the 2 K-groups), bf16 blocks (spline
    centers then silu/base) as [(c s p), o]."""
    ws = {}
    f8 = ml_dtypes.float8_e4m3
    for l in range(4):
        cfg = LCFG[l]
        din, dout = WIDTH[l], WIDTH[l + 1]
        coef = np.asarray(inp[f"coef{l}"], dtype=np.float32)
        sb = np.asarray(inp[f"sb{l}"], dtype=np.float32)
        ss = np.asarray(inp[f"ss{l}"], dtype=np.float32)
        spline_w = coef * ss[:, :, None] * (ALPHA / 6.0)  # [din, dout, 8]
        nic = din // 128
        gsel = [g for pr in cfg["pairs"] for g in pr]
        sp = spline_w[:, :, gsel]                      # [din, dout, 2*np]
        sp = sp.reshape(nic, 128, dout // 128, 128, cfg["np"], 2)
        # -> [c, j, p, oc, i, o128] then flatten cols (oc, i, o128)
        sp = sp.transpose(0, 4, 1, 2, 5, 3).reshape(nic * cfg["np"] * 128,
                                                    2 * dout)
        np.clip(sp, -240.0, 240.0, out=sp)
        ws[f"w8_{l}"] = np.ascontiguousarray(sp).astype(f8)
        bsel = spline_w[:, :, cfg["bf"]]               # [din, dout, ns-1]
        bb = np.concatenate([bsel, (sb * ALPHA)[:, :, None]], axis=2)
        bb = bb.reshape(nic, 128, dout, cfg["ns"]).transpose(0, 3, 1, 2)
        ws[f"wb_{l}"] = np.ascontiguousarray(
            bb.reshape(nic * cfg["ns"] * 128, dout)
        ).astype(ml_dtypes.bfloat16)
    bias = np.empty((128, 9), dtype=np.float32)
    bias[:, :8] = (3.5 - np.arange(8, dtype=np.float32))[None, :]
    bias[:, 8] = 2.0
    ws["bias_c"] = bias
    return ws


def _postprocess(core_out):
    """Device output is feature-major [out, bpc] and ALPHA-scaled."""
    return (np.asarray(core_out, dtype=np.float32) * INV_ALPHA).T


def _run(inputs, trace=False, **kwargs):
    inp = {k: np.asarray(v) for k, v in inputs.items()}
    ws = _build_weights(inp)
    x = np.concatenate(
        [inp["inputs_y"].astype(np.float32), inp["inputs_u"].astype(np.float32)],
        axis=1,
    )
    xT = np.ascontiguousarray(x.T)  # [512 feat, 4096 batch]
    nc = _get_nc()
    in_maps = []
    for c in range(NCORES):
        m = {"xT": np.ascontiguousarray(xT[:, c * BPC : (c + 1) * BPC])}
        m.update(ws)
        in_maps.append(m)
    res = run_bass_kernel_spmd(
        nc, in_maps, core_ids=list(range(NCORES)), trace=trace, **kwargs
    )
    out = np.concatenate(
        [_postprocess(r["out"]) for r in res.results], axis=0
    )
    return out, res


def kernel(**inputs) -> np.ndarray:
    out, _ = _run(inputs)
    return out


# revision 51
# speedup vs baseline: 1.0171x; 1.0171x over previous
"""KAN EncoderNetwork kernel for 8 Trainium2 NeuronCores.

Strategy (data-parallel, batch sharded 8 ways, weights replicated):

Each KAN layer  out = silu(x) @ sb + einsum('big,iog->bo', B(x), coef*ss)
is reformulated as matmuls over an expanded feature matrix: for every
128-wide input chunk the feature rows are spline basis blocks + 1 silu
block.  The uniform-grid cubic B-spline basis has the closed form
(cardinal spline, t = 2.5x + 5.5):

  6*B_g(x) = relu(2-w)^3 - 4*relu(1-w)^3,   w = |2.5x + 3.5 - g|

computed on ScalarE (Abs/Relu) + custom VectorE ops, balanced across the
two engines.  Everything stays feature-major ([feat, batch]) so layer
outputs in PSUM feed the next layer's basis computation directly; only
the final layer is transposed back (TensorE) for the [batch, out] output.

Precision/throughput structure (validated vs the fp32 reference on the
host, total rel err ~1e-2 vs the 2e-2 budget):
 * activations shrink layer by layer (x.std 0.5 -> 0.1), so deep-layer
   basis energy concentrates in the center blocks g=3,4: edge blocks are
   numerically dead and are DROPPED (L1: g 0,7; L2/L3: g 0,1,6,7).
 * center blocks g=3,4 stay bf16 on deep layers (they carry most of the
   spline signal); remaining blocks run as fp8e4m3 DoubleRow matmuls
   (2 K-tiles per instruction, ~2x PE throughput).  L0's activations are
   wide, so all 8 of its blocks tolerate fp8.
 * the silu/base block stays bf16 everywhere.
All weights are pre-scaled by ALPHA=2^15 so the tiny spline weights sit
in fp8's normal range; consumers of a layer's PSUM fold the 1/ALPHA
descale into their activation scale/s0 constants (no extra ops).
"""

import sys

sys.path.insert(0, "/opt/trn_rl_repo")

import numpy as np
import ml_dtypes

import concourse.bacc as bacc
import concourse.mybir as mybir
import concourse.tile as tile
from concourse.bass_utils import run_bass_kernel_spmd
from concourse.dve_spec import Spec, Src0, Src1, C0, C1, C2, Zero, relu, sq, maxx, lower, _has_src1
from concourse.dve_uop import DveOpSpec
from concourse.dve_ops import (
    DveOp,
    OPS,
    _SUB_OPCODE_FOR_NAME,
    CUSTOM_DVE_SPECS,
    _CUSTOM_DVE_ROW_BASE,
)

F32 = mybir.dt.float32
BF16 = mybir.dt.bfloat16
FP8 = mybir.dt.float8e4
AF = mybir.ActivationFunctionType
DR = mybir.MatmulPerfMode.DoubleRow

WIDTH = [512, 1024, 1024, 1024, 256]
NCORES = 8
BATCH = 4096
BPC = BATCH // NCORES  # 512 batch rows per core

ALPHA = float(2 ** 15)  # weight pre-scale so fp8 spline weights are normal
INV_ALPHA = 1.0 / ALPHA

# per-layer block config: fp8 DoubleRow pairs, bf16 spline blocks, and
# which blocks run on the ScalarE Abs/Relu pipeline (vb) vs all-DVE
LCFG = [
    dict(pairs=[(0, 1), (2, 3), (4, 5), (6, 7)], bf=[], vb={5, 6, 7}),
    dict(pairs=[(1, 6), (2, 5), (3, 4)], bf=[], vb={2, 3, 4}),
    dict(pairs=[(2, 5)], bf=[3, 4], vb={4, 5}),
    dict(pairs=[(2, 5)], bf=[3, 4], vb={4, 5}),
]
for _c in LCFG:
    _c["np"] = len(_c["pairs"])          # DoubleRow matmuls per chunk
    _c["ns"] = len(_c["bf"]) + 1         # bf16 matmuls per chunk (+silu)
    _c["slots"] = _c["np"] + _c["ns"]    # total matmul slots per chunk


def _register_op(name, spec):
    if name in _SUB_OPCODE_FOR_NAME:
        for op in OPS:
            if op.name == name:
                return op
        raise RuntimeError(f"opcode row taken but op {name} missing")
    row = _CUSTOM_DVE_ROW_BASE + len(OPS)
    _SUB_OPCODE_FOR_NAME[name] = row
    shas = {}
    for ver in ("v3", "v4"):
        uops = lower(spec, ver=ver)
        shas[ver] = DveOpSpec(
            name=name, opcode=row, uops=uops, rd1_en=_has_src1(spec)
        ).sha(ver)
    op = DveOp(name, spec, subdim=False, uops_sha=shas)
    OPS.append(op)
    CUSTOM_DVE_SPECS[name] = spec
    return op


# out = a^3 + s1 * relu(a - s0)^3   (in0 = a2 = relu(2-w); 1 stream)
_rb = relu(Src0 - C0)
KAN_TENT_POLY = _register_op(
    "KAN_TENT_POLY",
    Spec(
        body=sq(Src0) * Src0 + sq(_rb) * _rb * C1,
        reference=lambda in0, in1, s0, s1, imm2: in0**3
        + s1 * np.maximum(in0 - s0, 0.0) ** 3,
    ),
)

# a2 = relu(imm2 - |x*s0 + s1|)    (variant E pass 1; 1 stream, from x)
_u = Src0 * C0 + C1
_wabs = maxx(_u, Zero - _u)
KAN_A2_ABS = _register_op(
    "KAN_A2_ABS",
    Spec(
        body=relu(C2 - _wabs),
        reference=lambda in0, in1, s0, s1, imm2: np.maximum(
            imm2 - np.abs(in0 * s0 + s1), 0.0
        ),
    ),
)


def _build_nc():
    nc = bacc.Bacc(trn_type="TRN2")
    xT_dr = nc.dram_tensor("xT", [WIDTH[0], BPC], F32, kind="ExternalInput")
    w8_dr = [
        nc.dram_tensor(
            f"w8_{l}",
            [(WIDTH[l] // 128) * LCFG[l]["np"] * 128, 2 * WIDTH[l + 1]],
            FP8, kind="ExternalInput",
        )
        for l in range(4)
    ]
    wb_dr = [
        nc.dram_tensor(
            f"wb_{l}",
            [(WIDTH[l] // 128) * LCFG[l]["ns"] * 128, WIDTH[l + 1]],
            BF16, kind="ExternalInput",
        )
        for l in range(4)
    ]
    bias_dr = nc.dram_tensor("bias_c", [128, 9], F32, kind="ExternalInput")
    # feature-major ALPHA-scaled bf16 output; host transposes + descales
    out_dr = nc.dram_tensor("out", [WIDTH[4], BPC], BF16,
                            kind="ExternalOutput")

    # [p, c, j, w] where w = (oc, i, o128) flat: all of a chunk's
    # DoubleRow pairs land in ONE DMA, and any oc-aligned column phase is
    # a contiguous w-slice (keeps the DMA AP rank at 3)
    w8_r = [
        w8_dr[l].rearrange("(c j p) w -> p c j w", p=128, j=LCFG[l]["np"])
        for l in range(4)
    ]
    wb_r = [wb_dr[l].rearrange("(k p) o -> p k o", p=128) for l in range(4)]

    with tile.TileContext(nc) as tc:
        with (
            tc.tile_pool(name="const", bufs=1) as const_pool,
            tc.tile_pool(name="xt", bufs=2) as xt_pool,
            tc.tile_pool(name="ft", bufs=11) as ft_pool,
            tc.tile_pool(name="wt", bufs=10) as wt_pool,
            tc.tile_pool(name="tmp", bufs=3) as tmp_pool,
            tc.tile_pool(name="outp", bufs=1) as out_pool,
            tc.tile_pool(name="psum", bufs=8, space="PSUM") as psum_pool,
        ):
            nic0 = WIDTH[0] // 128
            xT_r = xT_dr.rearrange("(c p) b -> p c b", p=128)
            # DMA order = first-use order: the sync queue drains serially
            # and the FIRST dma only lands ~10us in (queue prologue + DGE
            # pipe), so everything the startup chain needs goes first.
            xt = xt_pool.tile([128, nic0, BPC], F32, tag="xt")
            nc.sync.dma_start(xt[:, 0:1, :], xT_r[:, 0:1, :])
            # col g in 0..7: Abs bias 3.5-g ; col 8: +2.0 (variant-B Relu
            # bias).  DMA'd, not memset: gpsimd's ~5us prologue would gate
            # the first ScalarE basis ops otherwise.
            bias = const_pool.tile([128, 9], F32, tag="bias")
            nc.sync.dma_start(bias, bias_dr.ap())
            pre_wt = wt_pool.tile([128, LCFG[0]["np"], 4, 2, 128], FP8,
                                  tag="wt8pre", bufs=1, name="wt_pre")
            nc.sync.dma_start(pre_wt, w8_r[0][:, 0, :, 0:1024])
            # warm-up operand, filled by DMA (content irrelevant)
            junk = const_pool.tile([128, BPC], F32, tag="junk")
            nc.sync.dma_start(junk, xT_r[:, 0, :])
            pre_wb = wt_pool.tile([128, 1, WIDTH[1]], BF16, tag="wtb",
                                  name="wb_pre")
            nc.sync.dma_start(pre_wb, wb_r[0][:, 0:1, :])
            nc.sync.dma_start(xt[:, 1:nic0, :], xT_r[:, 1:nic0, :])

            def new_ft(l, c):
                cfg = LCFG[l]
                ft = ft_pool.tile([128, 2 * cfg["np"], BPC], FP8, tag="ft8",
                                  name=f"ft_{l}_{c}")
                fb = ft_pool.tile([128, cfg["ns"], BPC], BF16, tag="ftb",
                                  name=f"fb_{l}_{c}")
                return ft, fb

            def emit_fast_restart(l, src_psum):
                """First fp8 block of chunk 0 computed straight from the
                previous layer's PSUM (holds ALPHA*x) so the PE restarts
                quickly."""
                g0 = LCFG[l]["pairs"][0][0]
                a2 = tmp_pool.tile([128, BPC], F32, tag="wv",
                                   name=f"a2fr_{l}")
                nc.vector._custom_dve(KAN_A2_ABS, out=a2, in0=src_psum,
                                      s0=2.5 * INV_ALPHA, s1=3.5 - g0,
                                      imm2=2.0)
                ft, fb = new_ft(l, 0)
                nc.vector._custom_dve(KAN_TENT_POLY, out=ft[:, 0, :],
                                      in0=a2, s0=1.0, s1=-4.0)
                return ft, fb

            def emit_copies(xt, src_psums, chunks):
                for i, c in enumerate(chunks):
                    if i % 2 == 0:
                        nc.scalar.copy(xt[:, c, :], src_psums[c])
                    else:
                        nc.vector.tensor_copy(xt[:, c, :], src_psums[c])

            def emit_basis(l, xt, c, ft, fb, skip_first=False):
                """ft: fp8 spline pairs (slot 2j+i = block pairs[j][i]);
                fb: bf16 blocks (cfg['bf'] order, then silu).
                fp8 pairs stage their a2 in bf16 and share ONE merged
                KAN_TENT_POLY (the DR matmul consumes both slots at once,
                so merging costs no pipeline granularity); bf16 center
                blocks keep f32 a2 for precision.
                For l>0 the input xt holds ALPHA*x."""
                cfg = LCFG[l]
                ds = INV_ALPHA if l > 0 else 1.0
                xa = xt[:, c, :]

                def a2_for(g, dst):
                    if g in cfg["vb"]:
                        wv = tmp_pool.tile([128, BPC], F32, tag="wv",
                                           name=f"wv_{l}_{c}_{g}")
                        nc.scalar.activation(wv, xa, AF.Abs,
                                             bias=bias[:, g : g + 1],
                                             scale=2.5 * ds)
                        nc.scalar.activation(dst, wv, AF.Relu,
                                             bias=bias[:, 8:9], scale=-1.0)
                    else:
                        nc.vector._custom_dve(KAN_A2_ABS, out=dst, in0=xa,
                                              s0=2.5 * ds, s1=3.5 - g,
                                              imm2=2.0)

                for j, pr in enumerate(cfg["pairs"]):
                    a2p = tmp_pool.tile([128, 2, BPC], BF16, tag="a2p",
                                        name=f"a2p_{l}_{c}_{j}")
                    lo = 1 if (skip_first and j == 0) else 0
                    for i in range(lo, 2):
                        a2_for(pr[i], a2p[:, i, :])
                    nc.vector._custom_dve(
                        KAN_TENT_POLY, out=ft[:, 2 * j + lo : 2 * j + 2, :],
                        in0=a2p[:, lo:2, :], s0=1.0, s1=-4.0,
                    )
                for s, g in enumerate(cfg["bf"]):
                    a2 = tmp_pool.tile([128, BPC], F32, tag="qv",
                                       name=f"a2_{l}_{c}_{g}")
                    a2_for(g, a2)
                    nc.vector._custom_dve(KAN_TENT_POLY, out=fb[:, s, :],
                                          in0=a2, s0=1.0, s1=-4.0)
                nc.scalar.activation(fb[:, cfg["ns"] - 1, :], xa, AF.Silu,
                                     scale=ds)

            def dma_weights(l, c, ocs, col0):
                """One DMA per weight class per chunk, covering all of
                `ocs` columns (sub-phases slice the same tile)."""
                cfg = LCFG[l]
                ncol = len(ocs) * 128
                if l == 0 and c == 0 and col0 == 0:
                    return pre_wt, pre_wb
                wt = wt_pool.tile([128, cfg["np"], len(ocs), 2, 128], FP8,
                                  tag="wt8", name=f"wt_{l}_{c}_{col0}")
                nc.sync.dma_start(
                    wt, w8_r[l][:, c, :, 2 * col0 : 2 * (col0 + ncol)]
                )
                wb = wt_pool.tile([128, cfg["ns"], ncol], BF16, tag="wtb",
                                  name=f"wb_{l}_{c}_{col0}")
                nc.sync.dma_start(
                    wb, wb_r[l][:, c * cfg["ns"] : (c + 1) * cfg["ns"],
                                col0 : col0 + ncol]
                )
                return wt, wb

            def emit_mms(l, c, wt, wb, ft, fb, psums, ocs, col0, KB):
                """col0 = base column of the weight tiles."""
                cfg = LCFG[l]
                ob = col0 // 128
                for j in range(cfg["np"]):
                    kb = c * cfg["slots"] + j
                    for oc in ocs:
                        nc.tensor.matmul(
                            psums[oc], wt[:, j, oc - ob, :, :],
                            ft[:, 2 * j : 2 * j + 2, :],
                            start=(kb == 0), stop=False, perf_mode=DR,
                        )
                for s in range(cfg["ns"]):
                    kb = c * cfg["slots"] + cfg["np"] + s
                    for oc in ocs:
                        nc.tensor.matmul(
                            psums[oc],
                            wb[:, s, (oc - ob) * 128 : (oc - ob + 1) * 128],
                            fb[:, s, :],
                            start=False, stop=(kb == KB - 1),
                        )

            # ---- layer 0: out-chunk phase split (4 + 4 banks) so layer-1
            # basis production fully overlaps phase B matmuls ----
            nicl0, nocl0 = WIDTH[0] // 128, WIDTH[1] // 128
            KB0 = LCFG[0]["slots"] * nicl0
            psums0 = [
                psum_pool.tile([128, BPC], F32, tag="psum", name=f"ps_0_{i}")
                for i in range(4)
            ]

            # HAM warm-up: slow fp32 matmuls at scheduler priority 0 so
            # they sit at the head of the PE queue and keep the array busy
            # through the ~3.4us HAM window while the first basis chain
            # ramps; the real kb==0 start=True matmul clears the bank.
            with tc.high_priority():
                for wi in range(2):
                    nc.tensor.matmul(
                        psums0[3], junk[:, 0:128], junk,
                        start=True, stop=True, skip_group_check=True,
                    )
            l0_fts = []
            w0A = []
            for c in range(nicl0):
                ft, fb = new_ft(0, c)
                emit_basis(0, xt, c, ft, fb)
                l0_fts.append((ft, fb))
                w0A.append(dma_weights(0, c, [0, 1, 2, 3], 0))
                emit_mms(0, c, *w0A[c], ft, fb, psums0, [0, 1], 0, KB0)

            # banks 0,1 done: layer-1 chunks 0,1 start while L0-A finishes
            nic1, noc1 = WIDTH[1] // 128, WIDTH[2] // 128
            KB1 = LCFG[1]["slots"] * nic1
            xt1 = xt_pool.tile([128, nic1, BPC], F32, tag="xt", name="xt_1")
            ft1_0, fb1_0 = emit_fast_restart(1, psums0[0])
            emit_copies(xt1, psums0, [0, 1])
            l1_fts = [(ft1_0, fb1_0)]
            emit_basis(1, xt1, 0, ft1_0, fb1_0, skip_first=True)
            ft, fb = new_ft(1, 1)
            emit_basis(1, xt1, 1, ft, fb)
            l1_fts.append((ft, fb))

            for c in range(nicl0):
                emit_mms(0, c, *w0A[c], *l0_fts[c], psums0, [2, 3], 0, KB0)
            emit_copies(xt1, psums0, [2, 3])
            psums1 = [
                psum_pool.tile([128, BPC], F32, tag="psum", name=f"ps_1_{i}")
                for i in range(4)
            ]
            for c in (2, 3):
                ft, fb = new_ft(1, c)
                emit_basis(1, xt1, c, ft, fb)
                l1_fts.append((ft, fb))

            # layer-0 phase B: weights for banks 4-7 DMA'd once per chunk;
            # matmuls run in oc-pair sub-phases so layer-1's late input
            # chunks materialize (and their basis runs) before phase B ends
            psums0 += [
                psum_pool.tile([128, BPC], F32, tag="psum", name=f"ps_0_{i}")
                for i in range(4, nocl0)
            ]
            w0B = [dma_weights(0, c, [4, 5, 6, 7], 512)
                   for c in range(nicl0)]
            for c in range(nicl0):
                emit_mms(0, c, *w0B[c], *l0_fts[c], psums0, [4, 5], 512, KB0)
            emit_copies(xt1, psums0, [4, 5])
            for c in (4, 5):
                ft, fb = new_ft(1, c)
                emit_basis(1, xt1, c, ft, fb)
                l1_fts.append((ft, fb))
            for c in range(nicl0):
                emit_mms(0, c, *w0B[c], *l0_fts[c], psums0, [6, 7], 512, KB0)
            emit_copies(xt1, psums0, [6, 7])
            psums1 += [
                psum_pool.tile([128, BPC], F32, tag="psum", name=f"ps_1_{i}")
                for i in range(4, noc1)
            ]
            for c in (6, 7):
                ft, fb = new_ft(1, c)
                emit_basis(1, xt1, c, ft, fb)
                l1_fts.append((ft, fb))

            # keep-warm filler: bridges the feature-starved boundary so
            # HAM does not re-throttle the PE
            nc.tensor.matmul(psums1[3], junk[:, 0:128], junk,
                             start=True, stop=True, skip_group_check=True)
            # layer-1 phase A (production already done above), sub-phased
            nic2, noc2 = WIDTH[2] // 128, WIDTH[3] // 128
            KB2 = LCFG[2]["slots"] * nic2
            w1A = [dma_weights(1, c, [0, 1, 2, 3], 0) for c in range(nic1)]
            for c in range(nic1):
                emit_mms(1, c, *w1A[c], *l1_fts[c], psums1, [0, 1], 0, KB1)

            # banks 0,1 done: layer-2 chunks 0,1 start while L1-A finishes
            xt2 = xt_pool.tile([128, nic2, BPC], F32, tag="xt", name="xt_2")
            ft2_0, fb2_0 = emit_fast_restart(2, psums1[0])
            emit_copies(xt2, psums1, [0, 1])
            l2_fts = [(ft2_0, fb2_0)]
            emit_basis(2, xt2, 0, ft2_0, fb2_0, skip_first=True)
            ft, fb = new_ft(2, 1)
            emit_basis(2, xt2, 1, ft, fb)
            l2_fts.append((ft, fb))

            for c in range(nic1):
                emit_mms(1, c, *w1A[c], *l1_fts[c], psums1, [2, 3], 0, KB1)
            emit_copies(xt2, psums1, [2, 3])
            psums2 = [
                psum_pool.tile([128, BPC], F32, tag="psum", name=f"ps_2_{i}")
                for i in range(4)
            ]
            for c in (2, 3):
                ft, fb = new_ft(2, c)
                emit_basis(2, xt2, c, ft, fb)
                l2_fts.append((ft, fb))

            # layer-1 phase B (oc-pair sub-phases, one weight DMA per chunk)
            w1B = [dma_weights(1, c, [4, 5, 6, 7], 512) for c in range(nic1)]
            for c in range(nic1):
                emit_mms(1, c, *w1B[c], *l1_fts[c], psums1, [4, 5], 512, KB1)
            emit_copies(xt2, psums1, [4, 5])
            for c in (4, 5):
                ft, fb = new_ft(2, c)
                emit_basis(2, xt2, c, ft, fb)
                l2_fts.append((ft, fb))
            for c in range(nic1):
                emit_mms(1, c, *w1B[c], *l1_fts[c], psums1, [6, 7], 512, KB1)
            emit_copies(xt2, psums1, [6, 7])
            psums2 += [
                psum_pool.tile([128, BPC], F32, tag="psum", name=f"ps_2_{i}")
                for i in range(4, noc2)
            ]
            for c in (6, 7):
                ft, fb = new_ft(2, c)
                emit_basis(2, xt2, c, ft, fb)
                l2_fts.append((ft, fb))

            nc.tensor.matmul(psums2[3], junk[:, 0:128], junk,
                             start=True, stop=True, skip_group_check=True)
            # layer-2 phase A (production already done above)
            for c in range(nic2):
                wt, wb = dma_weights(2, c, [0, 1, 2, 3], 0)
                emit_mms(2, c, wt, wb, *l2_fts[c], psums2,
                         [0, 1, 2, 3], 0, KB2)

            # between phases: layer-3 input chunks 0..3 + their basis
            nic3, noc3 = WIDTH[3] // 128, WIDTH[4] // 128
            KB3 = LCFG[3]["slots"] * nic3
            xt3 = xt_pool.tile([128, nic3, BPC], F32, tag="xt", name="xt_3")
            ft3_0, fb3_0 = emit_fast_restart(3, psums2[0])
            emit_copies(xt3, psums2, range(4))
            psums3 = [
                psum_pool.tile([128, BPC], F32, tag="psum", name=f"ps_3_{i}")
                for i in range(noc3)
            ]
            l3_fts = [(ft3_0, fb3_0)]
            emit_basis(3, xt3, 0, ft3_0, fb3_0, skip_first=True)
            for c in range(1, 4):
                ft, fb = new_ft(3, c)
                emit_basis(3, xt3, c, ft, fb)
                l3_fts.append((ft, fb))

            # layer-2 phase B (oc-pair sub-phases, one weight DMA per chunk)
            w2B = [dma_weights(2, c, [4, 5, 6, 7], 512) for c in range(nic2)]
            for c in range(nic2):
                emit_mms(2, c, *w2B[c], *l2_fts[c], psums2, [4, 5], 512, KB2)
            emit_copies(xt3, psums2, [4, 5])
            for c in (4, 5):
                ft, fb = new_ft(3, c)
                emit_basis(3, xt3, c, ft, fb)
                l3_fts.append((ft, fb))
            for c in range(nic2):
                emit_mms(2, c, *w2B[c], *l2_fts[c], psums2, [6, 7], 512, KB2)
            emit_copies(xt3, psums2, [6, 7])
            for c in (6, 7):
                ft, fb = new_ft(3, c)
                emit_basis(3, xt3, c, ft, fb)
                l3_fts.append((ft, fb))

            # layer-3 matmuls, chunk-major (late chunks last, so they hit
            # the PE right after their basis lands); output stays
            # feature-major ALPHA-scaled, host transposes + descales.
            s3 = out_pool.tile([128, noc3, BPC], BF16, tag="s3")
            out_r = out_dr.rearrange("(oc p) b -> p oc b", p=128)
            for c in range(nic3):
                wt, wb = dma_weights(3, c, [0, 1], 0)
                emit_mms(3, c, wt, wb, *l3_fts[c], psums3, [0, 1], 0, KB3)
            nc.scalar.copy(s3[:, 0, :], psums3[0])
            nc.vector.tensor_copy(s3[:, 1, :], psums3[1])
            for oc in range(noc3):
                nc.sync.dma_start(out_r[:, oc : oc + 1, :],
                                  s3[:, oc : oc + 1, :])
    nc.finalize()
    return nc


_NC_CACHE = []


def _get_nc():
    if not _NC_CACHE:
        _NC_CACHE.append(_build_nc())
    return _NC_CACHE[0]


def _build_weights(inp):
    """Pre-scale by ALPHA and pack per LCFG: fp8 DoubleRow pairs as
    [(c j p), (i o)] (i = which of the 2 K-groups), bf16 blocks (spline
    centers then silu/base) as [(c s p), o]."""
    ws = {}
    f8 = ml_dtypes.float8_e4m3
    for l in range(4):
        cfg = LCFG[l]
        din, dout = WIDTH[l], WIDTH[l + 1]
        coef = np.asarray(inp[f"coef{l}"], dtype=np.float32)
        sb = np.asarray(inp[f"sb{l}"], dtype=np.float32)
        ss = np.asarray(inp[f"ss{l}"], dtype=np.float32)
        spline_w = coef * ss[:, :, None] * (ALPHA / 6.0)  # [din, dout, 8]
        nic = din // 128
        gsel = [g for pr in cfg["pairs"] for g in pr]
        sp = spline_w[:, :, gsel]                      # [din, dout, 2*np]
        sp = sp.reshape(nic, 128, dout // 128, 128, cfg["np"], 2)
        # -> [c, j, p, oc, i, o128] then flatten cols (oc, i, o128)
        sp = sp.transpose(0, 4, 1, 2, 5, 3).reshape(nic * cfg["np"] * 128,
                                                    2 * dout)
        np.clip(sp, -240.0, 240.0, out=sp)
        ws[f"w8_{l}"] = np.ascontiguousarray(sp).astype(f8)
        bsel = spline_w[:, :, cfg["bf"]]               # [din, dout, ns-1]
        bb = np.concatenate([bsel, (sb * ALPHA)[:, :, None]], axis=2)
        bb = bb.reshape(nic, 128, dout, cfg["ns"]).transpose(0, 3, 1, 2)
        ws[f"wb_{l}"] = np.ascontiguousarray(
            bb.reshape(nic * cfg["ns"] * 128, dout)
        ).astype(ml_dtypes.bfloat16)
    bias = np.empty((128, 9), dtype=np.float32)
    bias[:, :8] = (3.5 - np.arange(8, dtype=np.float32))[None, :]
    bias[:, 8] = 2.0
    ws["bias_c"] = bias
    return ws


def _postprocess(core_out):
    """Device output is feature-major [out, bpc] and ALPHA-scaled."""
    return (np.asarray(core_out, dtype=np.float32) * INV_ALPHA).T


def _run(inputs, trace=False, **kwargs):
    inp = {k: np.asarray(v) for k, v in inputs.items()}
    ws = _build_weights(inp)
    x = np.concatenate(
        [inp["inputs_y"].astype(np.float32), inp["inputs_u"].astype(np.float32)],
        axis=1,
    )
    xT = np.ascontiguousarray(x.T)  # [512 feat, 4096 batch]
    nc = _get_nc()
    in_maps = []
    for c in range(NCORES):
        m = {"xT": np.ascontiguousarray(xT[:, c * BPC : (c + 1) * BPC])}
        m.update(ws)
        in_maps.append(m)
    res = run_bass_kernel_spmd(
        nc, in_maps, core_ids=list(range(NCORES)), trace=trace, **kwargs
    )
    out = np.concatenate(
        [_postprocess(r["out"]) for r in res.results], axis=0
    )
    return out, res


def kernel(**inputs) -> np.ndarray:
    out, _ = _run(inputs)
    return out


# revision 53
# speedup vs baseline: 1.0188x; 1.0017x over previous
"""KAN EncoderNetwork kernel for 8 Trainium2 NeuronCores.

Strategy (data-parallel, batch sharded 8 ways, weights replicated):

Each KAN layer  out = silu(x) @ sb + einsum('big,iog->bo', B(x), coef*ss)
is reformulated as matmuls over an expanded feature matrix: for every
128-wide input chunk the feature rows are spline basis blocks + 1 silu
block.  The uniform-grid cubic B-spline basis has the closed form
(cardinal spline, t = 2.5x + 5.5):

  6*B_g(x) = relu(2-w)^3 - 4*relu(1-w)^3,   w = |2.5x + 3.5 - g|

computed on ScalarE (Abs/Relu) + custom VectorE ops, balanced across the
two engines.  Everything stays feature-major ([feat, batch]) so layer
outputs in PSUM feed the next layer's basis computation directly; the
final output leaves the device feature-major and the host transposes.

Pipelining: each layer's out-banks are produced in oc-pair sub-phases so
the next layer's inputs (and their basis work) materialize while earlier
banks still accumulate; per-chunk weights arrive via ONE merged DMA per
class (the serialized DMA-trigger queue was a bottleneck); junk matmuls
bridge the two feature-starved phase boundaries so the PE's HAM clock
gate never re-throttles mid-kernel.

Precision/throughput structure (validated vs the fp32 reference on the
host, total rel err ~1e-2 vs the 2e-2 budget):
 * activations shrink layer by layer (x.std 0.5 -> 0.1), so deep-layer
   basis energy concentrates in the center blocks g=3,4: edge blocks are
   numerically dead and are DROPPED (L1: g 0,7; L2/L3: g 0,1,6,7).
 * center blocks g=3,4 stay bf16 on deep layers (they carry most of the
   spline signal); remaining blocks run as fp8e4m3 DoubleRow matmuls
   (2 K-tiles per instruction, ~2x PE throughput).  L0's activations are
   wide, so all 8 of its blocks tolerate fp8.
 * the silu/base block stays bf16 everywhere.
All weights are pre-scaled by ALPHA=2^15 so the tiny spline weights sit
in fp8's normal range; consumers of a layer's PSUM fold the 1/ALPHA
descale into their activation scale/s0 constants (no extra ops).
"""

import sys

sys.path.insert(0, "/opt/trn_rl_repo")

import numpy as np
import ml_dtypes

import concourse.bacc as bacc
import concourse.mybir as mybir
import concourse.tile as tile
from concourse.bass_utils import run_bass_kernel_spmd
from concourse.dve_spec import Spec, Src0, Src1, C0, C1, C2, Zero, relu, sq, maxx, lower, _has_src1
from concourse.dve_uop import DveOpSpec
from concourse.dve_ops import (
    DveOp,
    OPS,
    _SUB_OPCODE_FOR_NAME,
    CUSTOM_DVE_SPECS,
    _CUSTOM_DVE_ROW_BASE,
)

F32 = mybir.dt.float32
BF16 = mybir.dt.bfloat16
FP8 = mybir.dt.float8e4
AF = mybir.ActivationFunctionType
DR = mybir.MatmulPerfMode.DoubleRow

WIDTH = [512, 1024, 1024, 1024, 256]
NCORES = 8
BATCH = 4096
BPC = BATCH // NCORES  # 512 batch rows per core

ALPHA = float(2 ** 15)  # weight pre-scale so fp8 spline weights are normal
INV_ALPHA = 1.0 / ALPHA

# per-layer block config: fp8 DoubleRow pairs, bf16 spline blocks, and
# which blocks run on the ScalarE Abs/Relu pipeline (vb) vs all-DVE
LCFG = [
    dict(pairs=[(0, 1), (2, 3), (4, 5), (6, 7)], bf=[], vb={5, 6, 7}),
    dict(pairs=[(1, 6), (2, 5), (3, 4)], bf=[], vb={2, 3, 4}),
    dict(pairs=[(2, 5)], bf=[3, 4], vb={4, 5}),
    dict(pairs=[(2, 5)], bf=[3, 4], vb={4, 5}),
]
for _c in LCFG:
    _c["np"] = len(_c["pairs"])          # DoubleRow matmuls per chunk
    _c["ns"] = len(_c["bf"]) + 1         # bf16 matmuls per chunk (+silu)
    _c["slots"] = _c["np"] + _c["ns"]    # total matmul slots per chunk


def _register_op(name, spec):
    if name in _SUB_OPCODE_FOR_NAME:
        for op in OPS:
            if op.name == name:
                return op
        raise RuntimeError(f"opcode row taken but op {name} missing")
    row = _CUSTOM_DVE_ROW_BASE + len(OPS)
    _SUB_OPCODE_FOR_NAME[name] = row
    shas = {}
    for ver in ("v3", "v4"):
        uops = lower(spec, ver=ver)
        shas[ver] = DveOpSpec(
            name=name, opcode=row, uops=uops, rd1_en=_has_src1(spec)
        ).sha(ver)
    op = DveOp(name, spec, subdim=False, uops_sha=shas)
    OPS.append(op)
    CUSTOM_DVE_SPECS[name] = spec
    return op


# out = a^3 + s1 * relu(a - s0)^3   (in0 = a2 = relu(2-w); 1 stream)
_rb = relu(Src0 - C0)
KAN_TENT_POLY = _register_op(
    "KAN_TENT_POLY",
    Spec(
        body=sq(Src0) * Src0 + sq(_rb) * _rb * C1,
        reference=lambda in0, in1, s0, s1, imm2: in0**3
        + s1 * np.maximum(in0 - s0, 0.0) ** 3,
    ),
)

# a2 = relu(imm2 - |x*s0 + s1|)    (variant E pass 1; 1 stream, from x)
_u = Src0 * C0 + C1
_wabs = maxx(_u, Zero - _u)
KAN_A2_ABS = _register_op(
    "KAN_A2_ABS",
    Spec(
        body=relu(C2 - _wabs),
        reference=lambda in0, in1, s0, s1, imm2: np.maximum(
            imm2 - np.abs(in0 * s0 + s1), 0.0
        ),
    ),
)


def _build_nc():
    nc = bacc.Bacc(trn_type="TRN2")
    xT_dr = nc.dram_tensor("xT", [WIDTH[0], BPC], F32, kind="ExternalInput")
    w8_dr = [
        nc.dram_tensor(
            f"w8_{l}",
            [(WIDTH[l] // 128) * LCFG[l]["np"] * 128, 2 * WIDTH[l + 1]],
            FP8, kind="ExternalInput",
        )
        for l in range(4)
    ]
    wb_dr = [
        nc.dram_tensor(
            f"wb_{l}",
            [(WIDTH[l] // 128) * LCFG[l]["ns"] * 128, WIDTH[l + 1]],
            BF16, kind="ExternalInput",
        )
        for l in range(4)
    ]
    bias_dr = nc.dram_tensor("bias_c", [128, 9], F32, kind="ExternalInput")
    # feature-major ALPHA-scaled bf16 output; host transposes + descales
    out_dr = nc.dram_tensor("out", [WIDTH[4], BPC], BF16,
                            kind="ExternalOutput")

    # [p, c, j, w] where w = (oc, i, o128) flat: all of a chunk's
    # DoubleRow pairs land in ONE DMA, and any oc-aligned column phase is
    # a contiguous w-slice (keeps the DMA AP rank at 3)
    w8_r = [
        w8_dr[l].rearrange("(c j p) w -> p c j w", p=128, j=LCFG[l]["np"])
        for l in range(4)
    ]
    wb_r = [wb_dr[l].rearrange("(k p) o -> p k o", p=128) for l in range(4)]

    with tile.TileContext(nc) as tc:
        with (
            tc.tile_pool(name="const", bufs=1) as const_pool,
            tc.tile_pool(name="xt", bufs=2) as xt_pool,
            tc.tile_pool(name="ft", bufs=11) as ft_pool,
            tc.tile_pool(name="wt", bufs=10) as wt_pool,
            tc.tile_pool(name="tmp", bufs=3) as tmp_pool,
            tc.tile_pool(name="outp", bufs=1) as out_pool,
            tc.tile_pool(name="psum", bufs=8, space="PSUM") as psum_pool,
        ):
            nic0 = WIDTH[0] // 128
            xT_r = xT_dr.rearrange("(c p) b -> p c b", p=128)
            # DMA order = first-use order: the sync queue drains serially
            # and the FIRST dma only lands ~10us in (queue prologue + DGE
            # pipe), so everything the startup chain needs goes first.
            xt = xt_pool.tile([128, nic0, BPC], F32, tag="xt")
            nc.sync.dma_start(xt[:, 0:1, :], xT_r[:, 0:1, :])
            # col g in 0..7: Abs bias 3.5-g ; col 8: +2.0 (variant-B Relu
            # bias).  DMA'd, not memset: gpsimd's ~5us prologue would gate
            # the first ScalarE basis ops otherwise.
            bias = const_pool.tile([128, 9], F32, tag="bias")
            nc.sync.dma_start(bias, bias_dr.ap())
            pre_wt = wt_pool.tile([128, LCFG[0]["np"], 4, 2, 128], FP8,
                                  tag="wt8pre", bufs=1, name="wt_pre")
            nc.sync.dma_start(pre_wt, w8_r[0][:, 0, :, 0:1024])
            # warm-up operand, filled by DMA (content irrelevant)
            junk = const_pool.tile([128, BPC], F32, tag="junk")
            nc.sync.dma_start(junk, xT_r[:, 0, :])
            pre_wb = wt_pool.tile([128, 1, WIDTH[1]], BF16, tag="wtb",
                                  name="wb_pre")
            nc.sync.dma_start(pre_wb, wb_r[0][:, 0:1, :])
            nc.sync.dma_start(xt[:, 1:nic0, :], xT_r[:, 1:nic0, :])

            def new_ft(l, c):
                cfg = LCFG[l]
                ft = ft_pool.tile([128, 2 * cfg["np"], BPC], FP8, tag="ft8",
                                  name=f"ft_{l}_{c}")
                fb = ft_pool.tile([128, cfg["ns"], BPC], BF16, tag="ftb",
                                  name=f"fb_{l}_{c}")
                return ft, fb

            def emit_fast_restart(l, src_psum):
                """First fp8 block of chunk 0 computed straight from the
                previous layer's PSUM (holds ALPHA*x) so the PE restarts
                quickly."""
                g0 = LCFG[l]["pairs"][0][0]
                a2 = tmp_pool.tile([128, BPC], F32, tag="wv",
                                   name=f"a2fr_{l}")
                nc.vector._custom_dve(KAN_A2_ABS, out=a2, in0=src_psum,
                                      s0=2.5 * INV_ALPHA, s1=3.5 - g0,
                                      imm2=2.0)
                ft, fb = new_ft(l, 0)
                nc.vector._custom_dve(KAN_TENT_POLY, out=ft[:, 0, :],
                                      in0=a2, s0=1.0, s1=-4.0)
                return ft, fb

            def emit_copies(xt, src_psums, chunks):
                for i, c in enumerate(chunks):
                    if i % 2 == 0:
                        nc.scalar.copy(xt[:, c, :], src_psums[c])
                    else:
                        nc.vector.tensor_copy(xt[:, c, :], src_psums[c])

            def emit_basis(l, xt, c, ft, fb, skip_first=False):
                """ft: fp8 spline pairs (slot 2j+i = block pairs[j][i]);
                fb: bf16 blocks (cfg['bf'] order, then silu).
                fp8 pairs stage their a2 in bf16 and share ONE merged
                KAN_TENT_POLY (the DR matmul consumes both slots at once,
                so merging costs no pipeline granularity); bf16 center
                blocks keep f32 a2 for precision.
                For l>0 the input xt holds ALPHA*x."""
                cfg = LCFG[l]
                ds = INV_ALPHA if l > 0 else 1.0
                xa = xt[:, c, :]

                def a2_for(g, dst):
                    if g in cfg["vb"]:
                        wv = tmp_pool.tile([128, BPC], F32, tag="wv",
                                           name=f"wv_{l}_{c}_{g}")
                        nc.scalar.activation(wv, xa, AF.Abs,
                                             bias=bias[:, g : g + 1],
                                             scale=2.5 * ds)
                        nc.scalar.activation(dst, wv, AF.Relu,
                                             bias=bias[:, 8:9], scale=-1.0)
                    else:
                        nc.vector._custom_dve(KAN_A2_ABS, out=dst, in0=xa,
                                              s0=2.5 * ds, s1=3.5 - g,
                                              imm2=2.0)

                for j, pr in enumerate(cfg["pairs"]):
                    a2p = tmp_pool.tile([128, 2, BPC], BF16, tag="a2p",
                                        name=f"a2p_{l}_{c}_{j}")
                    lo = 1 if (skip_first and j == 0) else 0
                    for i in range(lo, 2):
                        a2_for(pr[i], a2p[:, i, :])
                    nc.vector._custom_dve(
                        KAN_TENT_POLY, out=ft[:, 2 * j + lo : 2 * j + 2, :],
                        in0=a2p[:, lo:2, :], s0=1.0, s1=-4.0,
                    )
                for s, g in enumerate(cfg["bf"]):
                    a2 = tmp_pool.tile([128, BPC], F32, tag="qv",
                                       name=f"a2_{l}_{c}_{g}")
                    a2_for(g, a2)
                    nc.vector._custom_dve(KAN_TENT_POLY, out=fb[:, s, :],
                                          in0=a2, s0=1.0, s1=-4.0)
                nc.scalar.activation(fb[:, cfg["ns"] - 1, :], xa, AF.Silu,
                                     scale=ds)

            def dma_weights(l, c, ocs, col0):
                """One DMA per weight class per chunk, covering all of
                `ocs` columns (sub-phases slice the same tile)."""
                cfg = LCFG[l]
                ncol = len(ocs) * 128
                if l == 0 and c == 0 and col0 == 0:
                    return pre_wt, pre_wb
                wt = wt_pool.tile([128, cfg["np"], len(ocs), 2, 128], FP8,
                                  tag="wt8", name=f"wt_{l}_{c}_{col0}")
                nc.sync.dma_start(
                    wt, w8_r[l][:, c, :, 2 * col0 : 2 * (col0 + ncol)]
                )
                wb = wt_pool.tile([128, cfg["ns"], ncol], BF16, tag="wtb",
                                  name=f"wb_{l}_{c}_{col0}")
                nc.sync.dma_start(
                    wb, wb_r[l][:, c * cfg["ns"] : (c + 1) * cfg["ns"],
                                col0 : col0 + ncol]
                )
                return wt, wb

            def emit_mms(l, c, wt, wb, ft, fb, psums, ocs, col0, KB):
                """col0 = base column of the weight tiles."""
                cfg = LCFG[l]
                ob = col0 // 128
                for j in range(cfg["np"]):
                    kb = c * cfg["slots"] + j
                    for oc in ocs:
                        nc.tensor.matmul(
                            psums[oc], wt[:, j, oc - ob, :, :],
                            ft[:, 2 * j : 2 * j + 2, :],
                            start=(kb == 0), stop=False, perf_mode=DR,
                        )
                for s in range(cfg["ns"]):
                    kb = c * cfg["slots"] + cfg["np"] + s
                    for oc in ocs:
                        nc.tensor.matmul(
                            psums[oc],
                            wb[:, s, (oc - ob) * 128 : (oc - ob + 1) * 128],
                            fb[:, s, :],
                            start=False, stop=(kb == KB - 1),
                        )

            # ---- layer 0: out-chunk phase split (4 + 4 banks) so layer-1
            # basis production fully overlaps phase B matmuls ----
            nicl0, nocl0 = WIDTH[0] // 128, WIDTH[1] // 128
            KB0 = LCFG[0]["slots"] * nicl0
            psums0 = [
                psum_pool.tile([128, BPC], F32, tag="psum", name=f"ps_0_{i}")
                for i in range(4)
            ]

            # HAM warm-up: slow fp32 matmuls at scheduler priority 0 so
            # they sit at the head of the PE queue and keep the array busy
            # through the ~3.4us HAM window while the first basis chain
            # ramps; the real kb==0 start=True matmul clears the bank.
            with tc.high_priority():
                for wi in range(2):
                    nc.tensor.matmul(
                        psums0[3], junk[:, 0:128], junk,
                        start=True, stop=True, skip_group_check=True,
                    )
            l0_fts = []
            w0A = []
            for c in range(nicl0):
                ft, fb = new_ft(0, c)
                emit_basis(0, xt, c, ft, fb)
                l0_fts.append((ft, fb))
                w0A.append(dma_weights(0, c, [0, 1, 2, 3], 0))
                emit_mms(0, c, *w0A[c], ft, fb, psums0, [0, 1], 0, KB0)

            # banks 0,1 done: layer-1 chunks 0,1 start while L0-A finishes
            nic1, noc1 = WIDTH[1] // 128, WIDTH[2] // 128
            KB1 = LCFG[1]["slots"] * nic1
            xt1 = xt_pool.tile([128, nic1, BPC], F32, tag="xt", name="xt_1")
            ft1_0, fb1_0 = emit_fast_restart(1, psums0[0])
            emit_copies(xt1, psums0, [0, 1])
            l1_fts = [(ft1_0, fb1_0)]
            emit_basis(1, xt1, 0, ft1_0, fb1_0, skip_first=True)
            ft, fb = new_ft(1, 1)
            emit_basis(1, xt1, 1, ft, fb)
            l1_fts.append((ft, fb))

            for c in range(nicl0):
                emit_mms(0, c, *w0A[c], *l0_fts[c], psums0, [2, 3], 0, KB0)
            emit_copies(xt1, psums0, [2, 3])
            psums1 = [
                psum_pool.tile([128, BPC], F32, tag="psum", name=f"ps_1_{i}")
                for i in range(4)
            ]
            for c in (2, 3):
                ft, fb = new_ft(1, c)
                emit_basis(1, xt1, c, ft, fb)
                l1_fts.append((ft, fb))

            # layer-0 phase B: weights for banks 4-7 DMA'd once per chunk;
            # matmuls run in oc-pair sub-phases so layer-1's late input
            # chunks materialize (and their basis runs) before phase B ends
            psums0 += [
                psum_pool.tile([128, BPC], F32, tag="psum", name=f"ps_0_{i}")
                for i in range(4, nocl0)
            ]
            w0B = [dma_weights(0, c, [4, 5, 6, 7], 512)
                   for c in range(nicl0)]
            for c in range(nicl0):
                emit_mms(0, c, *w0B[c], *l0_fts[c], psums0, [4, 5], 512, KB0)
            emit_copies(xt1, psums0, [4, 5])
            for c in (4, 5):
                ft, fb = new_ft(1, c)
                emit_basis(1, xt1, c, ft, fb)
                l1_fts.append((ft, fb))
            for c in range(nicl0):
                emit_mms(0, c, *w0B[c], *l0_fts[c], psums0, [6], 512, KB0)
            emit_copies(xt1, psums0, [6])
            ft, fb = new_ft(1, 6)
            emit_basis(1, xt1, 6, ft, fb)
            l1_fts.append((ft, fb))
            for c in range(nicl0):
                emit_mms(0, c, *w0B[c], *l0_fts[c], psums0, [7], 512, KB0)
            emit_copies(xt1, psums0, [7])
            psums1 += [
                psum_pool.tile([128, BPC], F32, tag="psum", name=f"ps_1_{i}")
                for i in range(4, noc1)
            ]
            ft, fb = new_ft(1, 7)
            emit_basis(1, xt1, 7, ft, fb)
            l1_fts.append((ft, fb))

            # keep-warm filler: bridges the feature-starved boundary so
            # HAM does not re-throttle the PE
            nc.tensor.matmul(psums1[3], junk[:, 0:128], junk,
                             start=True, stop=True, skip_group_check=True)
            # layer-1 phase A (production already done above), sub-phased
            nic2, noc2 = WIDTH[2] // 128, WIDTH[3] // 128
            KB2 = LCFG[2]["slots"] * nic2
            w1A = [dma_weights(1, c, [0, 1, 2, 3], 0) for c in range(nic1)]
            for c in range(nic1):
                emit_mms(1, c, *w1A[c], *l1_fts[c], psums1, [0, 1], 0, KB1)

            # banks 0,1 done: layer-2 chunks 0,1 start while L1-A finishes
            xt2 = xt_pool.tile([128, nic2, BPC], F32, tag="xt", name="xt_2")
            ft2_0, fb2_0 = emit_fast_restart(2, psums1[0])
            emit_copies(xt2, psums1, [0, 1])
            l2_fts = [(ft2_0, fb2_0)]
            emit_basis(2, xt2, 0, ft2_0, fb2_0, skip_first=True)
            ft, fb = new_ft(2, 1)
            emit_basis(2, xt2, 1, ft, fb)
            l2_fts.append((ft, fb))

            for c in range(nic1):
                emit_mms(1, c, *w1A[c], *l1_fts[c], psums1, [2, 3], 0, KB1)
            emit_copies(xt2, psums1, [2, 3])
            psums2 = [
                psum_pool.tile([128, BPC], F32, tag="psum", name=f"ps_2_{i}")
                for i in range(4)
            ]
            for c in (2, 3):
                ft, fb = new_ft(2, c)
                emit_basis(2, xt2, c, ft, fb)
                l2_fts.append((ft, fb))

            # layer-1 phase B (oc-pair sub-phases, one weight DMA per chunk)
            w1B = [dma_weights(1, c, [4, 5, 6, 7], 512) for c in range(nic1)]
            for c in range(nic1):
                emit_mms(1, c, *w1B[c], *l1_fts[c], psums1, [4, 5], 512, KB1)
            emit_copies(xt2, psums1, [4, 5])
            for c in (4, 5):
                ft, fb = new_ft(2, c)
                emit_basis(2, xt2, c, ft, fb)
                l2_fts.append((ft, fb))
            for c in range(nic1):
                emit_mms(1, c, *w1B[c], *l1_fts[c], psums1, [6, 7], 512, KB1)
            emit_copies(xt2, psums1, [6, 7])
            psums2 += [
                psum_pool.tile([128, BPC], F32, tag="psum", name=f"ps_2_{i}")
                for i in range(4, noc2)
            ]
            for c in (6, 7):
                ft, fb = new_ft(2, c)
                emit_basis(2, xt2, c, ft, fb)
                l2_fts.append((ft, fb))

            nc.tensor.matmul(psums2[3], junk[:, 0:128], junk,
                             start=True, stop=True, skip_group_check=True)
            # layer-2 phase A (production already done above)
            for c in range(nic2):
                wt, wb = dma_weights(2, c, [0, 1, 2, 3], 0)
                emit_mms(2, c, wt, wb, *l2_fts[c], psums2,
                         [0, 1, 2, 3], 0, KB2)

            # between phases: layer-3 input chunks 0..3 + their basis
            nic3, noc3 = WIDTH[3] // 128, WIDTH[4] // 128
            KB3 = LCFG[3]["slots"] * nic3
            xt3 = xt_pool.tile([128, nic3, BPC], F32, tag="xt", name="xt_3")
            ft3_0, fb3_0 = emit_fast_restart(3, psums2[0])
            emit_copies(xt3, psums2, range(4))
            psums3 = [
                psum_pool.tile([128, BPC], F32, tag="psum", name=f"ps_3_{i}")
                for i in range(noc3)
            ]
            l3_fts = [(ft3_0, fb3_0)]
            emit_basis(3, xt3, 0, ft3_0, fb3_0, skip_first=True)
            for c in range(1, 4):
                ft, fb = new_ft(3, c)
                emit_basis(3, xt3, c, ft, fb)
                l3_fts.append((ft, fb))

            # layer-2 phase B (oc-pair sub-phases, one weight DMA per chunk)
            w2B = [dma_weights(2, c, [4, 5, 6, 7], 512) for c in range(nic2)]
            for c in range(nic2):
                emit_mms(2, c, *w2B[c], *l2_fts[c], psums2, [4, 5], 512, KB2)
            emit_copies(xt3, psums2, [4, 5])
            for c in (4, 5):
                ft, fb = new_ft(3, c)
                emit_basis(3, xt3, c, ft, fb)
                l3_fts.append((ft, fb))
            for c in range(nic2):
                emit_mms(2, c, *w2B[c], *l2_fts[c], psums2, [6], 512, KB2)
            emit_copies(xt3, psums2, [6])
            ft, fb = new_ft(3, 6)
            emit_basis(3, xt3, 6, ft, fb)
            l3_fts.append((ft, fb))
            for c in range(nic2):
                emit_mms(2, c, *w2B[c], *l2_fts[c], psums2, [7], 512, KB2)
            emit_copies(xt3, psums2, [7])
            ft, fb = new_ft(3, 7)
            emit_basis(3, xt3, 7, ft, fb)
            l3_fts.append((ft, fb))

            # layer-3 matmuls, chunk-major (late chunks last, so they hit
            # the PE right after their basis lands); output stays
            # feature-major ALPHA-scaled, host transposes + descales.
            s3 = out_pool.tile([128, noc3, BPC], BF16, tag="s3")
            out_r = out_dr.rearrange("(oc p) b -> p oc b", p=128)
            for c in range(nic3):
                wt, wb = dma_weights(3, c, [0, 1], 0)
                emit_mms(3, c, wt, wb, *l3_fts[c], psums3, [0, 1], 0, KB3)
            nc.scalar.copy(s3[:, 0, :], psums3[0])
            nc.vector.tensor_copy(s3[:, 1, :], psums3[1])
            for oc in range(noc3):
                nc.sync.dma_start(out_r[:, oc : oc + 1, :],
                                  s3[:, oc : oc + 1, :])
    nc.finalize()
    return nc


_NC_CACHE = []


def _get_nc():
    if not _NC_CACHE:
        _NC_CACHE.append(_build_nc())
    return _NC_CACHE[0]


def _build_weights(inp):
    """Pre-scale by ALPHA and pack per LCFG: fp8 DoubleRow pairs as
    [(c j p), (i o)] (i = which of the 2 K-groups), bf16 blocks (spline
    centers then silu/base) as [(c s p), o]."""
    ws = {}
    f8 = ml_dtypes.float8_e4m3
    for l in range(4):
        cfg = LCFG[l]
        din, dout = WIDTH[l], WIDTH[l + 1]
        coef = np.asarray(inp[f"coef{l}"], dtype=np.float32)
        sb = np.asarray(inp[f"sb{l}"], dtype=np.float32)
        ss = np.asarray(inp[f"ss{l}"], dtype=np.float32)
        spline_w = coef * ss[:, :, None] * (ALPHA / 6.0)  # [din, dout, 8]
        nic = din // 128
        gsel = [g for pr in cfg["pairs"] for g in pr]
        sp = spline_w[:, :, gsel]                      # [din, dout, 2*np]
        sp = sp.reshape(nic, 128, dout // 128, 128, cfg["np"], 2)
        # -> [c, j, p, oc, i, o128] then flatten cols (oc, i, o128)
        sp = sp.transpose(0, 4, 1, 2, 5, 3).reshape(nic * cfg["np"] * 128,
                                                    2 * dout)
        np.clip(sp, -240.0, 240.0, out=sp)
        ws[f"w8_{l}"] = np.ascontiguousarray(sp).astype(f8)
        bsel = spline_w[:, :, cfg["bf"]]               # [din, dout, ns-1]
        bb = np.concatenate([bsel, (sb * ALPHA)[:, :, None]], axis=2)
        bb = bb.reshape(nic, 128, dout, cfg["ns"]).transpose(0, 3, 1, 2)
        ws[f"wb_{l}"] = np.ascontiguousarray(
            bb.reshape(nic * cfg["ns"] * 128, dout)
        ).astype(ml_dtypes.bfloat16)
    bias = np.empty((128, 9), dtype=np.float32)
    bias[:, :8] = (3.5 - np.arange(8, dtype=np.float32))[None, :]
    bias[:, 8] = 2.0
    ws["bias_c"] = bias
    return ws


def _postprocess(core_out):
    """Device output is feature-major [out, bpc] and ALPHA-scaled."""
    return (np.asarray(core_out, dtype=np.float32) * INV_ALPHA).T


def _run(inputs, trace=False, **kwargs):
    inp = {k: np.asarray(v) for k, v in inputs.items()}
    ws = _build_weights(inp)
    x = np.concatenate(
        [inp["inputs_y"].astype(np.float32), inp["inputs_u"].astype(np.float32)],
        axis=1,
    )
    xT = np.ascontiguousarray(x.T)  # [512 feat, 4096 batch]
    nc = _get_nc()
    in_maps = []
    for c in range(NCORES):
        m = {"xT": np.ascontiguousarray(xT[:, c * BPC : (c + 1) * BPC])}
        m.update(ws)
        in_maps.append(m)
    res = run_bass_kernel_spmd(
        nc, in_maps, core_ids=list(range(NCORES)), trace=trace, **kwargs
    )
    out = np.concatenate(
        [_postprocess(r["out"]) for r in res.results], axis=0
    )
    return out, res


def kernel(**inputs) -> np.ndarray:
    out, _ = _run(inputs)
    return out
